# revision 57
# baseline (speedup 1.0000x reference)
"""GQA sliding-window attention (training path, no causal mask, no 1/sqrt(d)
scaling) on 8 Trainium2 NeuronCores.

Reference semantics (see original nn.Module):
  q = x@Wq+bq [b,s,16,64]; k,v = x@Wk+bk / x@Wv+bv [b,s,2,64]
  k,v zero-padded by 128 on both sides of s; query i attends padded
  positions [i, i+256) (i.e. global [i-128, i+128)); padded positions
  contribute score 0 (exp->1) and value 0. out = attn @ Wo + bo.

Sharding: batch x sequence. 8 shards = 2 batches x 4 chunks of 512 query
rows. Each core receives x^T for its 512 rows plus a 128-row halo on each
side (zero rows outside [0, 2048)), with an appended 0/1 validity row so
that K/V bias is only added at in-range positions. Host gathers per-core
outputs; no collectives.

Per-core dataflow (bf16 matmul inputs, fp32 PSUM accumulation):
  K/V projected over the 768-row halo (bias via augmented weight row,
  emitted first so the PE starts on the early-arriving aug operands); V
  transposed back to natural layout with a ones column appended so the
  PV matmuls also produce the softmax denominator. qT per 128-dim block
  with ScalarE bias-add; host permutes Wq columns so block p holds head
  p (group 0) in rows 0:64 and head p+8 (group 1) in rows 64:128, and
  pre-blocks Wq/Wo so each 128x128 panel is one contiguous DMA.
  Attention runs in 8 pair-tiles (group, 128-row qblock): scores
  S^T[w 128, 8 heads, 128 q] as two N=512 matmuls per window chunk
  (3 chunks cover the 384-position window), exp on ScalarE, band-mask
  triangles via 2 GPSIMD affine_selects whose multi-dim pattern
  broadcasts over the packed heads, PV accumulates [65, 8, 128] with
  row 64 = denominator. Normalization: ScalarE moves the denominator
  row to SBUF, DVE reciprocal_approx_fast inverts it, GpSimd
  partition_broadcast replicates it across partitions, one DVE multiply
  rescales attnT; the chain is software-pipelined one tile deep
  (broadcast at next-tile start, multiply at next-tile end) and the
  last tile runs it split in head-halves to shorten the serial tail.
  Output projection with ScalarE bias-add, streamed to DRAM on three
  DMA queues.
"""

import numpy as np

DIM = 1024
NH = 16  # query heads
G = 2  # kv heads
HD = 64  # head dim
W = 256  # window
HALF = 128
BATCH, SEQ = 2, 2048
NCORES = 8
SQ = 512  # query rows per core
SK = SQ + 2 * HALF  # 768 kv halo rows per core
KC = DIM // 128  # 8 contraction chunks
NJ = SK // 128  # 6 kv chunks



_CACHE = {}


def _build_program(dbg=False):
    import concourse.bass as bass
    import concourse.mybir as mybir
    import concourse.tile as tile
    from concourse import bacc

    f32 = mybir.dt.float32
    f32r = mybir.dt.float32r
    bf16 = mybir.dt.bfloat16

    nc = bacc.Bacc("TRN2", target_bir_lowering=False, debug=False, num_devices=NCORES)
    dbg_t = {}
    if dbg:
        for name, shape, dt in [
            ("dbg_qT", [128, KC, SQ], bf16), ("dbg_kT", [128, SK], bf16),
            ("dbg_vT", [128, SK], f32r),
            ("dbg_pt0", [128, 3, 4, 128], bf16),
            ("dbg_attnT", [128, KC, SQ], bf16),
            ("dbg_pvden", [1, 4, 128], f32),
            ("dbg_recs", [1, 4, 128], f32),
        ]:
            dbg_t[name] = nc.declare_dram_parameter(name, shape, dt, isOutput=True)

    xaT = nc.declare_dram_parameter("xaT", [DIM + 1, SK], bf16, isOutput=False)
    wq = nc.declare_dram_parameter("wq", [KC, 128, KC, 128], bf16, isOutput=False)
    wk = nc.declare_dram_parameter("wk", [DIM + 1, G * HD], bf16, isOutput=False)
    wv = nc.declare_dram_parameter("wv", [DIM + 1, G * HD], bf16, isOutput=False)
    wo = nc.declare_dram_parameter("wo", [KC, 128, KC, 128], bf16, isOutput=False)
    bq = nc.declare_dram_parameter("bq", [DIM, 1], f32, isOutput=False)
    bo = nc.declare_dram_parameter("bo", [DIM, 1], f32, isOutput=False)
    identD = nc.declare_dram_parameter("ident", [128, 128], f32r, isOutput=False)
    ones2 = nc.declare_dram_parameter("ones2", [128, G], bf16, isOutput=False)
    yT = nc.declare_dram_parameter("yT", [DIM, SQ], f32, isOutput=True)

    with tile.TileContext(nc) as tc:
        with (
            nc.allow_low_precision("bf16 matmul inputs; accumulation stays fp32"),
            tc.tile_pool(name="wts", bufs=1) as wts,
            tc.tile_pool(name="sb", bufs=1) as sb,
            tc.tile_pool(name="pt", bufs=3) as ptp,
            tc.tile_pool(name="dr", bufs=6) as drp,
            tc.tile_pool(name="rbp", bufs=4) as rbp,
            tc.tile_pool(name="yst", bufs=2) as yst,
            tc.tile_pool(name="psS", bufs=2, space="PSUM") as psS,
            tc.tile_pool(name="pvP", bufs=2, space="PSUM") as pvP,
        ):
            # ---- constant loads (critical-path order: wk/wv, xT, wq, wo) ----
            # The aug rows ride at the very front of the fast HW queues so
            # the PE's first (aug) matmuls can issue almost immediately.
            xaug = wts.tile([1, SK], bf16, tag="xaug")
            nc.sync.dma_start(out=xaug[:, :], in_=xaT[DIM:DIM + 1, :])
            wk_aug = wts.tile([1, G * HD], bf16, tag="wkaug")
            wv_aug = wts.tile([1, G * HD], bf16, tag="wvaug")
            nc.sync.dma_start(out=wk_aug[:, :], in_=wk[DIM:DIM + 1, :])
            nc.scalar.dma_start(out=wv_aug[:, :], in_=wv[DIM:DIM + 1, :])
            qs = (nc.sync, nc.scalar)
            wk_sb = wts.tile([128, KC, G * HD], bf16, tag="wk")
            wv_sb = wts.tile([128, KC, G * HD], bf16, tag="wv")
            xT_sb = wts.tile([128, KC, SK], bf16, tag="xT")
            for kc in range(KC):
                eng = qs[kc % 2]
                eng.dma_start(out=wk_sb[:, kc, :],
                              in_=wk[kc * 128:(kc + 1) * 128, :])
                eng.dma_start(out=xT_sb[:, kc, :],
                              in_=xaT[kc * 128:(kc + 1) * 128, :])
            nc.gpsimd.dma_start(
                out=wv_sb[:, :, :],
                in_=wv[0:DIM, :].rearrange("(kc p) m -> p kc m", p=128))
            wq_sb = wts.tile([128, KC, KC, 128], bf16, tag="wq")
            for dd in range(KC):
                qs[dd % 2].dma_start(out=wq_sb[:, dd, :, :], in_=wq[dd, :, :, :])
            wo_sb = wts.tile([128, KC, KC, 128], bf16, tag="wo")
            for do in range(KC):
                qs[do % 2].dma_start(out=wo_sb[:, do, :, :], in_=wo[do, :, :, :])

            bq_sb = wts.tile([128, KC], f32, tag="bq")
            bo_sb = wts.tile([128, KC], f32, tag="bo")
            nc.gpsimd.dma_start(
                out=bq_sb[:, :], in_=bq.rearrange("(a p) c -> p (a c)", p=128))
            nc.gpsimd.dma_start(
                out=bo_sb[:, :], in_=bo.rearrange("(a p) c -> p (a c)", p=128))
            ident = wts.tile([128, 128], f32r, tag="ident")
            nc.gpsimd.dma_start(out=ident[:, :], in_=identD[:, :])
            ones_sb = wts.tile([128, G], bf16, tag="ones")
            nc.gpsimd.dma_start(out=ones_sb[:, :], in_=ones2[:, :])

            # ---- persistent intermediates ----
            qT_sb = sb.tile([128, KC, SQ], bf16, tag="qT")  # [dk(2 heads), dd, q]
            kT_sb = sb.tile([128, SK], bf16, tag="kT")      # [dk(2 groups), w]
            vT_sb = sb.tile([128, SK], f32r, tag="vT")
            vt_t = [
                sb.tile([128, G, HD + 1], bf16, tag=f"vt{j}", name=f"vt{j}")
                for j in range(NJ)
            ]
            attnT = sb.tile([128, KC, SQ], bf16, tag="attnT")  # [dk(2 heads), pair, q]

            # ---- K/V projections over the full 768 halo ----
            # The tiny aug bias-row matmul goes FIRST (its operands arrive on
            # the fast gpsimd queue) so the PE starts before wk/xT land.
            for (wmat, waug, dst) in ((wk_sb, wk_aug, kT_sb), (wv_sb, wv_aug, vT_sb)):
                for h2 in range(2):
                    ps = psS.tile([128, KC, 128], f32, tag="sc")
                    out = ps[:, 0:3, :].rearrange("p a b -> p (a b)")
                    sl = slice(h2 * 384, (h2 + 1) * 384)
                    nc.tensor.matmul(out, waug[:, :], xaug[:, sl],
                                     start=True, stop=False)
                    for kc in range(KC):
                        nc.tensor.matmul(
                            out, wmat[:, kc, :], xT_sb[:, kc, sl],
                            start=False, stop=(kc == KC - 1),
                        )
                    nc.vector.tensor_copy(dst[:, sl], out)

            # ---- V back to natural layout [w, dk], ones column appended ----
            for j in range(NJ):
                ps = psS.tile([128, KC, 128], f32r, tag="sc", name=f"pstr{j}")
                out = ps[:, 0, :]
                nc.tensor.transpose(out, vT_sb[:, j * 128:(j + 1) * 128], ident)
                nc.vector.tensor_copy(
                    vt_t[j][:, :, 0:HD],
                    out.rearrange("p (g d) -> p g d", g=G),
                )
                nc.vector.tensor_copy(vt_t[j][:, :, HD:HD + 1], ones_sb[:, :])

            def q_proj(dd):
                ps = psS.tile([128, KC, 128], f32, tag="sc", name=f"psq{dd}")
                out = ps[:, 0:4, :].rearrange("p a b -> p (a b)")
                for kc in range(KC):
                    nc.tensor.matmul(
                        out, wq_sb[:, dd, kc, :], xT_sb[:, kc, HALF:HALF + SQ],
                        start=(kc == 0), stop=(kc == KC - 1),
                    )
                nc.scalar.activation(
                    qT_sb[:, dd, :], out, mybir.ActivationFunctionType.Identity,
                    bias=bq_sb[:, dd:dd + 1],
                )

            if dbg:
                nc.sync.dma_start(out=dbg_t["dbg_kT"][:, :], in_=kT_sb[:, :])
                nc.sync.dma_start(out=dbg_t["dbg_vT"][:, :], in_=vT_sb[:, :])

            # ---- attention in 8-head pair-tiles ----
            # Tile (g, qb): all 8 heads of group g, local q block
            # [128qb, 128qb+128). Window chunks qb+c (c=0..2); band masks:
            # c=0 keeps ww-qq>=0, c=2 keeps qq-ww-1>=0, c=1 is full. Scores
            # and PV run as hh-half matmuls (moving free dim caps at 512)
            # into shared [128, 8, 128] PSUM tiles.
            for dd in range(KC):
                q_proj(dd)

            tiles = [(g, qb) for g in range(G) for qb in range(4)]

            def emit_bcast(p, half=None):
                # rb[d, h, q] = 1/den(h, q) on every partition; bf16
                # throughout so the broadcast and the normalization multiply
                # both run in the 2-byte fast paths
                g, qb, dr, ti = p
                rb = rbp.tile([128, KC, 128], bf16, tag="rb", name=f"rb{ti}_{half}")
                hs = slice(None) if half is None else slice(4 * half, 4 * half + 4)
                nc.gpsimd.partition_broadcast(rb[:, hs, :], dr[:, hs, :])
                return rb

            def emit_mul(p, rb, half=None):
                g, qb, dr, ti = p
                hs = slice(None) if half is None else slice(4 * half, 4 * half + 4)
                region = attnT[64 * g:64 * g + 64, hs, qb * 128:(qb + 1) * 128]
                nc.vector.tensor_mul(region, region, rb[64 * g:64 * g + 64, hs, :])

            prev = None
            for ti, (g, qb) in enumerate(tiles):
                last = ti == len(tiles) - 1
                pt = ptp.tile([128, 3, KC, 128], bf16, tag="pt")
                for c in range(3):
                    ps = psS.tile([128, KC, 128], f32, tag="sc")
                    for hh in range(2):
                        nc.tensor.matmul(
                            ps[:, 4 * hh:4 * hh + 4, :],
                            kT_sb[64 * g:64 * g + 64,
                                  (qb + c) * 128:(qb + c + 1) * 128],
                            qT_sb[64 * g:64 * g + 64, 4 * hh:4 * hh + 4,
                                  qb * 128:(qb + 1) * 128],
                            start=True, stop=True,
                        )
                    nc.scalar.activation(pt[:, c, :, :], ps[:, :, :],
                                         mybir.ActivationFunctionType.Exp)
                    if c == 0:
                        nc.gpsimd.affine_select(
                            out=pt[:, 0, :, :], in_=pt[:, 0, :, :],
                            compare_op=mybir.AluOpType.is_ge, fill=0.0,
                            base=0, channel_multiplier=1,
                            pattern=[[0, KC], [-1, 128]],
                        )
                    elif c == 2:
                        nc.gpsimd.affine_select(
                            out=pt[:, 2, :, :], in_=pt[:, 2, :, :],
                            compare_op=mybir.AluOpType.is_ge, fill=0.0,
                            base=-1, channel_multiplier=-1,
                            pattern=[[0, KC], [1, 128]],
                        )
                if dbg and (g, qb) == (0, 0):
                    nc.sync.dma_start(out=dbg_t["dbg_pt0"][:, :, :, :],
                                      in_=pt[:, :, 0:4, :])
                # previous tile's broadcast rides BEHIND this tile's masks
                # on the GpSimd FIFO: the masks gate this tile's PV and must
                # not queue behind a broadcast that is still waiting on the
                # previous tile's reciprocal; the broadcast itself is not
                # needed until this tile's end (deferred multiply)
                if prev is not None:
                    rb_prev = emit_bcast(prev)
                pv = pvP.tile([128, KC, 128], f32, tag="pv")
                for c in range(3):
                    for hh in range(2):
                        nc.tensor.matmul(
                            pv[0:HD + 1, 4 * hh:4 * hh + 4, :],
                            vt_t[qb + c][:, g, :],
                            pt[:, c, 4 * hh:4 * hh + 4, :],
                            start=(c == 0), stop=(c == 2),
                        )
                # reciprocal_approx_fast cannot read PSUM: ScalarE moves the
                # denominator row to SBUF first. dn/recip go ahead of the
                # attnT copy so the cross-engine norm chain is never gated
                # on it; the last tile runs the whole chain in head-halves
                # to shorten the serial latency before the output projection.
                dn = drp.tile([1, KC, 128], f32, tag="dn", name=f"dn{ti}")
                dr0 = drp.tile([1, KC, 128], f32, tag="dr0", name=f"dr0_{ti}")
                dr = drp.tile([1, KC, 128], bf16, tag="dr", name=f"dr{ti}")
                halves = (0, 1) if last else (None,)
                for hf in halves:
                    hs = slice(None) if hf is None else slice(4 * hf, 4 * hf + 4)
                    nc.scalar.activation(dn[:, hs, :], pv[HD:HD + 1, hs, :],
                                         mybir.ActivationFunctionType.Identity)
                    nc.vector.reciprocal_approx_fast(dr0[:, hs, :], dn[:, hs, :])
                    nc.vector.tensor_copy(dr[:, hs, :], dr0[:, hs, :])
                nc.vector.tensor_copy(
                    attnT[64 * g:64 * g + 64, :, qb * 128:(qb + 1) * 128],
                    pv[0:HD, :, :])
                if dbg and (g, qb) == (0, 0):
                    nc.sync.dma_start(out=dbg_t["dbg_pvden"][:, :, :],
                                      in_=dn[:, 0:4, :])
                    nc.sync.dma_start(out=dbg_t["dbg_recs"][:, :, :],
                                      in_=dr[:, 0:4, :])
                # previous tile's normalization multiply goes after this
                # tile's DVE work: its broadcast had the whole tile to land
                if prev is not None:
                    emit_mul(prev, rb_prev)
                prev = (g, qb, dr, ti)
            for hf in (0, 1):
                rb_last = emit_bcast(prev, hf)
                emit_mul(prev, rb_last, hf)

            if dbg:
                nc.sync.dma_start(out=dbg_t["dbg_qT"][:, :, :], in_=qT_sb[:, :, :])
                nc.sync.dma_start(out=dbg_t["dbg_attnT"][:, :, :], in_=attnT[:, :, :])

            # ---- output projection ----
            # q[0:384] only needs qblocks 0-2, whose normalizations land
            # well before the last tile's chain: those 24 matmuls per block
            # overlap the final-tile denominator path; the q[384:512] group
            # follows once the last multiplies finish.
            for do in range(KC):
                ps = psS.tile([128, KC, 128], f32, tag="sc")
                out = ps[:, 0:4, :].rearrange("p a b -> p (a b)")
                for p in range(KC):
                    nc.tensor.matmul(
                        out[:, 0:384], wo_sb[:, do, p, :], attnT[:, p, 0:384],
                        start=(p == 0), stop=(p == KC - 1),
                    )
                for p in range(KC):
                    nc.tensor.matmul(
                        out[:, 384:512], wo_sb[:, do, p, :],
                        attnT[:, p, 384:512],
                        start=(p == 0), stop=(p == KC - 1),
                    )
                yt = yst.tile([128, SQ], f32, tag="yt")
                nc.scalar.activation(yt, out, mybir.ActivationFunctionType.Identity,
                                     bias=bo_sb[:, do:do + 1])
                eng = (nc.sync, nc.scalar, nc.gpsimd)[do % 3]
                eng.dma_start(out=yT[do * 128:(do + 1) * 128, :], in_=yt[:, :])

    nc.finalize()
    return nc


def get_program(dbg=False):
    key = ("nc", dbg)
    if key not in _CACHE:
        _CACHE[key] = _build_program(dbg)
    return _CACHE[key]


def make_in_maps(x, Wq, bq, Wk, bk, Wv, bv, Wo, bo):
    """Host-side sharding: per-core input dicts."""
    import ml_dtypes

    bf16 = ml_dtypes.bfloat16
    x = np.ascontiguousarray(np.asarray(x, np.float32))
    wkb = np.concatenate([np.asarray(Wk, np.float32), np.asarray(bk, np.float32)[None]], 0)
    wvb = np.concatenate([np.asarray(Wv, np.float32), np.asarray(bv, np.float32)[None]], 0)
    # head permutation: device column-block p holds [head p | head p+8]
    perm = np.empty(DIM, np.int64)
    for p in range(8):
        perm[128 * p:128 * p + 64] = np.arange(64 * p, 64 * p + 64)
        perm[128 * p + 64:128 * p + 128] = np.arange(64 * (p + 8), 64 * (p + 8) + 64)
    wqp = np.asarray(Wq, np.float32)[:, perm]
    wop = np.asarray(Wo, np.float32)[perm, :]
    # [dd, part, kc, m] blocks so each dd's weights are one contiguous DMA
    wq_blk = np.ascontiguousarray(
        wqp.reshape(KC, 128, KC, 128).transpose(2, 1, 0, 3).astype(bf16))
    wo_blk = np.ascontiguousarray(
        wop.reshape(KC, 128, KC, 128).transpose(2, 1, 0, 3).astype(bf16))
    common = {
        "wq": wq_blk,
        "wk": np.ascontiguousarray(wkb.astype(bf16)),
        "wv": np.ascontiguousarray(wvb.astype(bf16)),
        "wo": wo_blk,
        "bq": np.ascontiguousarray(np.asarray(bq, np.float32)[perm].reshape(DIM, 1)),
        "bo": np.ascontiguousarray(np.asarray(bo, np.float32).reshape(DIM, 1)),
        "ident": np.eye(128, dtype=np.float32),
        "ones2": np.ones((128, G), bf16),
    }
    in_maps = []
    for c in range(NCORES):
        b, t = divmod(c, NCORES // BATCH)
        s0 = SQ * t
        xa = np.zeros((SK, DIM + 1), np.float32)
        lo, hi = max(0, s0 - HALF), min(SEQ, s0 + SQ + HALF)
        xa[lo - (s0 - HALF):hi - (s0 - HALF), :DIM] = x[b, lo:hi]
        xa[lo - (s0 - HALF):hi - (s0 - HALF), DIM] = 1.0
        in_maps.append({"xaT": np.ascontiguousarray(xa.T.astype(bf16)), **common})
    return in_maps


def assemble_output(results):
    y = np.empty((BATCH, SEQ, DIM), np.float32)
    for c in range(NCORES):
        b, t = divmod(c, NCORES // BATCH)
        y[b, SQ * t:SQ * (t + 1), :] = results[c]["yT"].T
    return y


def kernel(**inputs):
    from concourse.bass_utils import run_bass_kernel_spmd

    nc = get_program()
    in_maps = make_in_maps(**inputs)
    last_err = None
    for _ in range(3):  # retry: transient NRT device wedges recover on rerun
        try:
            res = run_bass_kernel_spmd(nc, in_maps, list(range(NCORES)))
            return assemble_output(res.results)
        except Exception as e:  # noqa: BLE001
            last_err = e
    raise last_err


# revision 58
# speedup vs baseline: 1.2177x; 1.2177x over previous
"""GQA sliding-window attention (training path, no causal mask, no 1/sqrt(d)
scaling) on 8 Trainium2 NeuronCores.

Reference semantics (see original nn.Module):
  q = x@Wq+bq [b,s,16,64]; k,v = x@Wk+bk / x@Wv+bv [b,s,2,64]
  k,v zero-padded by 128 on both sides of s; query i attends padded
  positions [i, i+256) (i.e. global [i-128, i+128)); padded positions
  contribute score 0 (exp->1) and value 0. out = attn @ Wo + bo.

Sharding: batch x sequence. 8 shards = 2 batches x 4 chunks of 512 query
rows. Each core receives x^T for its 512 rows plus a 128-row halo on each
side (zero rows outside [0, 2048)), with an appended 0/1 validity row so
that K/V bias is only added at in-range positions. Host gathers per-core
outputs; no collectives.

Per-core dataflow (bf16 matmul inputs, fp32 PSUM accumulation):
  K/V projected over the 768-row halo (bias via augmented weight row,
  emitted first so the PE starts on the early-arriving aug operands); V
  transposed back to natural layout with a ones column appended so the
  PV matmuls also produce the softmax denominator. qT per 128-dim block
  with ScalarE bias-add; host permutes Wq columns so block p holds head
  p (group 0) in rows 0:64 and head p+8 (group 1) in rows 64:128, and
  pre-blocks Wq/Wo so each 128x128 panel is one contiguous DMA.
  Attention runs in 8 pair-tiles (group, 128-row qblock): scores
  S^T[w 128, 8 heads, 128 q] as two N=512 matmuls per window chunk
  (3 chunks cover the 384-position window), exp on ScalarE, band-mask
  triangles via 2 GPSIMD affine_selects whose multi-dim pattern
  broadcasts over the packed heads, PV accumulates [65, 8, 128] with
  row 64 = denominator. Normalization: ScalarE moves the denominator
  row to SBUF, DVE reciprocal_approx_fast inverts it, GpSimd
  partition_broadcast replicates it across partitions, one DVE multiply
  rescales attnT; the chain is software-pipelined one tile deep
  (broadcast at next-tile start, multiply at next-tile end) and the
  last tile runs it split in head-halves to shorten the serial tail.
  Output projection with ScalarE bias-add, streamed to DRAM on three
  DMA queues.
"""

import numpy as np

DIM = 1024
NH = 16  # query heads
G = 2  # kv heads
HD = 64  # head dim
W = 256  # window
HALF = 128
BATCH, SEQ = 2, 2048
NCORES = 8
SQ = 512  # query rows per core
SK = SQ + 2 * HALF  # 768 kv halo rows per core
KC = DIM // 128  # 8 contraction chunks
NJ = SK // 128  # 6 kv chunks



_CACHE = {}


def _build_program(dbg=False):
    import concourse.bass as bass
    import concourse.mybir as mybir
    import concourse.tile as tile
    from concourse import bacc

    f32 = mybir.dt.float32
    f32r = mybir.dt.float32r
    bf16 = mybir.dt.bfloat16

    nc = bacc.Bacc("TRN2", target_bir_lowering=False, debug=False, num_devices=NCORES)
    dbg_t = {}
    if dbg:
        for name, shape, dt in [
            ("dbg_qT", [128, KC, SQ], bf16), ("dbg_kT", [128, SK], bf16),
            ("dbg_vT", [128, SK], f32r),
            ("dbg_pt0", [128, 3, 4, 128], bf16),
            ("dbg_attnT", [128, KC, SQ], bf16),
            ("dbg_pvden", [1, 4, 128], f32),
            ("dbg_recs", [1, 4, 128], f32),
        ]:
            dbg_t[name] = nc.declare_dram_parameter(name, shape, dt, isOutput=True)

    xaT = nc.declare_dram_parameter("xaT", [DIM + 1, SK], bf16, isOutput=False)
    wq = nc.declare_dram_parameter("wq", [KC, 128, KC, 128], bf16, isOutput=False)
    wk = nc.declare_dram_parameter("wk", [DIM + 1, G * HD], bf16, isOutput=False)
    wv = nc.declare_dram_parameter("wv", [DIM + 1, G * HD], bf16, isOutput=False)
    wo = nc.declare_dram_parameter("wo", [KC, 128, KC, 128], bf16, isOutput=False)
    bq = nc.declare_dram_parameter("bq", [DIM, 1], f32, isOutput=False)
    bo = nc.declare_dram_parameter("bo", [DIM, 1], f32, isOutput=False)
    identD = nc.declare_dram_parameter("ident", [128, 128], f32r, isOutput=False)
    ones2 = nc.declare_dram_parameter("ones2", [128, G], bf16, isOutput=False)
    yT = nc.declare_dram_parameter("yT", [DIM, SQ], f32, isOutput=True)

    with tile.TileContext(nc) as tc:
        with (
            nc.allow_low_precision("bf16 matmul inputs; accumulation stays fp32"),
            tc.tile_pool(name="wts", bufs=1) as wts,
            tc.tile_pool(name="sb", bufs=1) as sb,
            tc.tile_pool(name="pt", bufs=3) as ptp,
            tc.tile_pool(name="dr", bufs=6) as drp,
            tc.tile_pool(name="rbp", bufs=4) as rbp,
            tc.tile_pool(name="yst", bufs=2) as yst,
            tc.tile_pool(name="psS", bufs=2, space="PSUM") as psS,
            tc.tile_pool(name="pvP", bufs=2, space="PSUM") as pvP,
        ):
            # ---- constant loads (critical-path order: wk/wv, xT, wq, wo) ----
            # The aug rows ride at the very front of the fast HW queues so
            # the PE's first (aug) matmuls can issue almost immediately.
            xaug = wts.tile([1, SK], bf16, tag="xaug")
            nc.sync.dma_start(out=xaug[:, :], in_=xaT[DIM:DIM + 1, :])
            wk_aug = wts.tile([1, G * HD], bf16, tag="wkaug")
            wv_aug = wts.tile([1, G * HD], bf16, tag="wvaug")
            nc.sync.dma_start(out=wk_aug[:, :], in_=wk[DIM:DIM + 1, :])
            nc.scalar.dma_start(out=wv_aug[:, :], in_=wv[DIM:DIM + 1, :])
            qs = (nc.sync, nc.scalar)
            wk_sb = wts.tile([128, KC, G * HD], bf16, tag="wk")
            wv_sb = wts.tile([128, KC, G * HD], bf16, tag="wv")
            xT_sb = wts.tile([128, KC, SK], bf16, tag="xT")
            for kc in range(KC):
                eng = qs[kc % 2]
                eng.dma_start(out=wk_sb[:, kc, :],
                              in_=wk[kc * 128:(kc + 1) * 128, :])
                eng.dma_start(out=xT_sb[:, kc, :],
                              in_=xaT[kc * 128:(kc + 1) * 128, :])
            nc.gpsimd.dma_start(
                out=wv_sb[:, :, :],
                in_=wv[0:DIM, :].rearrange("(kc p) m -> p kc m", p=128))
            wq_sb = wts.tile([128, KC, KC, 128], bf16, tag="wq")
            for dd in range(KC):
                qs[dd % 2].dma_start(out=wq_sb[:, dd, :, :], in_=wq[dd, :, :, :])
            wo_sb = wts.tile([128, KC, KC, 128], bf16, tag="wo")
            for do in range(KC):
                qs[do % 2].dma_start(out=wo_sb[:, do, :, :], in_=wo[do, :, :, :])

            bq_sb = wts.tile([128, KC], f32, tag="bq")
            bo_sb = wts.tile([128, KC], f32, tag="bo")
            nc.gpsimd.dma_start(
                out=bq_sb[:, :], in_=bq.rearrange("(a p) c -> p (a c)", p=128))
            nc.gpsimd.dma_start(
                out=bo_sb[:, :], in_=bo.rearrange("(a p) c -> p (a c)", p=128))
            ident = wts.tile([128, 128], f32r, tag="ident")
            nc.gpsimd.dma_start(out=ident[:, :], in_=identD[:, :])
            ones_sb = wts.tile([128, G], bf16, tag="ones")
            nc.gpsimd.dma_start(out=ones_sb[:, :], in_=ones2[:, :])

            # ---- persistent intermediates ----
            qT_sb = sb.tile([128, KC, SQ], bf16, tag="qT")  # [dk(2 heads), dd, q]
            kT_sb = sb.tile([128, SK], bf16, tag="kT")      # [dk(2 groups), w]
            vT_sb = sb.tile([128, SK], f32r, tag="vT")
            vt_t = [
                sb.tile([128, G, HD + 1], bf16, tag=f"vt{j}", name=f"vt{j}")
                for j in range(NJ)
            ]
            attnT = sb.tile([128, KC, SQ], bf16, tag="attnT")  # [dk(2 heads), pair, q]

            # ---- K/V projections over the full 768 halo ----
            # The tiny aug bias-row matmul goes FIRST (its operands arrive on
            # the fast gpsimd queue) so the PE starts before wk/xT land.
            for (wmat, waug, dst) in ((wk_sb, wk_aug, kT_sb), (wv_sb, wv_aug, vT_sb)):
                for h2 in range(2):
                    ps = psS.tile([128, KC, 128], f32, tag="sc")
                    out = ps[:, 0:3, :].rearrange("p a b -> p (a b)")
                    sl = slice(h2 * 384, (h2 + 1) * 384)
                    nc.tensor.matmul(out, waug[:, :], xaug[:, sl],
                                     start=True, stop=False)
                    for kc in range(KC):
                        nc.tensor.matmul(
                            out, wmat[:, kc, :], xT_sb[:, kc, sl],
                            start=False, stop=(kc == KC - 1),
                        )
                    nc.vector.tensor_copy(dst[:, sl], out)

            # ---- V back to natural layout [w, dk], ones column appended ----
            for j in range(NJ):
                ps = psS.tile([128, KC, 128], f32r, tag="sc", name=f"pstr{j}")
                out = ps[:, 0, :]
                nc.tensor.transpose(out, vT_sb[:, j * 128:(j + 1) * 128], ident)
                nc.vector.tensor_copy(
                    vt_t[j][:, :, 0:HD],
                    out.rearrange("p (g d) -> p g d", g=G),
                )
                nc.vector.tensor_copy(vt_t[j][:, :, HD:HD + 1], ones_sb[:, :])

            def q_proj(dd):
                ps = psS.tile([128, KC, 128], f32, tag="sc", name=f"psq{dd}")
                out = ps[:, 0:4, :].rearrange("p a b -> p (a b)")
                for kc in range(KC):
                    nc.tensor.matmul(
                        out, wq_sb[:, dd, kc, :], xT_sb[:, kc, HALF:HALF + SQ],
                        start=(kc == 0), stop=(kc == KC - 1),
                    )
                nc.scalar.activation(
                    qT_sb[:, dd, :], out, mybir.ActivationFunctionType.Identity,
                    bias=bq_sb[:, dd:dd + 1],
                )

            if dbg:
                nc.sync.dma_start(out=dbg_t["dbg_kT"][:, :], in_=kT_sb[:, :])
                nc.sync.dma_start(out=dbg_t["dbg_vT"][:, :], in_=vT_sb[:, :])

            # ---- attention in 8-head pair-tiles ----
            # Tile (g, qb): all 8 heads of group g, local q block
            # [128qb, 128qb+128). Window chunks qb+c (c=0..2); band masks:
            # c=0 keeps ww-qq>=0, c=2 keeps qq-ww-1>=0, c=1 is full. Scores
            # and PV run as hh-half matmuls (moving free dim caps at 512)
            # into shared [128, 8, 128] PSUM tiles.
            for dd in range(KC):
                q_proj(dd)

            tiles = [(g, qb) for g in range(G) for qb in range(4)]

            def emit_bcast(p, half=None):
                # rb[d, h, q] = 1/den(h, q) on every partition
                g, qb, dr, ti = p
                rb = rbp.tile([128, KC, 128], f32, tag="rb", name=f"rb{ti}_{half}")
                hs = slice(None) if half is None else slice(4 * half, 4 * half + 4)
                nc.gpsimd.partition_broadcast(rb[:, hs, :], dr[:, hs, :])
                return rb

            def emit_mul(p, rb, half=None):
                g, qb, dr, ti = p
                hs = slice(None) if half is None else slice(4 * half, 4 * half + 4)
                region = attnT[64 * g:64 * g + 64, hs, qb * 128:(qb + 1) * 128]
                nc.vector.tensor_mul(region, region, rb[64 * g:64 * g + 64, hs, :])

            prev = None
            for ti, (g, qb) in enumerate(tiles):
                last = ti == len(tiles) - 1
                pt = ptp.tile([128, 3, KC, 128], bf16, tag="pt")
                for c in range(3):
                    ps = psS.tile([128, KC, 128], f32, tag="sc")
                    for hh in range(2):
                        nc.tensor.matmul(
                            ps[:, 4 * hh:4 * hh + 4, :],
                            kT_sb[64 * g:64 * g + 64,
                                  (qb + c) * 128:(qb + c + 1) * 128],
                            qT_sb[64 * g:64 * g + 64, 4 * hh:4 * hh + 4,
                                  qb * 128:(qb + 1) * 128],
                            start=True, stop=True,
                        )
                    nc.scalar.activation(pt[:, c, :, :], ps[:, :, :],
                                         mybir.ActivationFunctionType.Exp)
                    if c == 0:
                        nc.gpsimd.affine_select(
                            out=pt[:, 0, :, :], in_=pt[:, 0, :, :],
                            compare_op=mybir.AluOpType.is_ge, fill=0.0,
                            base=0, channel_multiplier=1,
                            pattern=[[0, KC], [-1, 128]],
                        )
                    elif c == 2:
                        nc.gpsimd.affine_select(
                            out=pt[:, 2, :, :], in_=pt[:, 2, :, :],
                            compare_op=mybir.AluOpType.is_ge, fill=0.0,
                            base=-1, channel_multiplier=-1,
                            pattern=[[0, KC], [1, 128]],
                        )
                if dbg and (g, qb) == (0, 0):
                    nc.sync.dma_start(out=dbg_t["dbg_pt0"][:, :, :, :],
                                      in_=pt[:, :, 0:4, :])
                # previous tile's broadcast rides BEHIND this tile's masks
                # on the GpSimd FIFO: the masks gate this tile's PV and must
                # not queue behind a broadcast that is still waiting on the
                # previous tile's reciprocal; the broadcast itself is not
                # needed until this tile's end (deferred multiply)
                if prev is not None:
                    rb_prev = emit_bcast(prev)
                pv = pvP.tile([128, KC, 128], f32, tag="pv")
                for c in range(3):
                    for hh in range(2):
                        nc.tensor.matmul(
                            pv[0:HD + 1, 4 * hh:4 * hh + 4, :],
                            vt_t[qb + c][:, g, :],
                            pt[:, c, 4 * hh:4 * hh + 4, :],
                            start=(c == 0), stop=(c == 2),
                        )
                # reciprocal_approx_fast cannot read PSUM: ScalarE moves the
                # denominator row to SBUF first. dn/recip go ahead of the
                # attnT copy so the cross-engine norm chain is never gated
                # on it; the last tile runs the whole chain in head-halves
                # to shorten the serial latency before the output projection.
                dn = drp.tile([1, KC, 128], f32, tag="dn", name=f"dn{ti}")
                dr = drp.tile([1, KC, 128], f32, tag="dr", name=f"dr{ti}")
                halves = (0, 1) if last else (None,)
                for hf in halves:
                    hs = slice(None) if hf is None else slice(4 * hf, 4 * hf + 4)
                    nc.scalar.activation(dn[:, hs, :], pv[HD:HD + 1, hs, :],
                                         mybir.ActivationFunctionType.Identity)
                    nc.vector.reciprocal_approx_fast(dr[:, hs, :], dn[:, hs, :])
                nc.vector.tensor_copy(
                    attnT[64 * g:64 * g + 64, :, qb * 128:(qb + 1) * 128],
                    pv[0:HD, :, :])
                if dbg and (g, qb) == (0, 0):
                    nc.sync.dma_start(out=dbg_t["dbg_pvden"][:, :, :],
                                      in_=dn[:, 0:4, :])
                    nc.sync.dma_start(out=dbg_t["dbg_recs"][:, :, :],
                                      in_=dr[:, 0:4, :])
                # previous tile's normalization multiply goes after this
                # tile's DVE work: its broadcast had the whole tile to land
                if prev is not None:
                    emit_mul(prev, rb_prev)
                prev = (g, qb, dr, ti)
            for hf in (0, 1):
                rb_last = emit_bcast(prev, hf)
                emit_mul(prev, rb_last, hf)

            if dbg:
                nc.sync.dma_start(out=dbg_t["dbg_qT"][:, :, :], in_=qT_sb[:, :, :])
                nc.sync.dma_start(out=dbg_t["dbg_attnT"][:, :, :], in_=attnT[:, :, :])

            # ---- output projection ----
            # q[0:384] only needs qblocks 0-2, whose normalizations land
            # well before the last tile's chain: those 24 matmuls per block
            # overlap the final-tile denominator path; the q[384:512] group
            # follows once the last multiplies finish.
            for do in range(KC):
                ps = psS.tile([128, KC, 128], f32, tag="sc")
                out = ps[:, 0:4, :].rearrange("p a b -> p (a b)")
                for p in range(KC):
                    nc.tensor.matmul(
                        out[:, 0:384], wo_sb[:, do, p, :], attnT[:, p, 0:384],
                        start=(p == 0), stop=(p == KC - 1),
                    )
                for p in range(KC):
                    nc.tensor.matmul(
                        out[:, 384:512], wo_sb[:, do, p, :],
                        attnT[:, p, 384:512],
                        start=(p == 0), stop=(p == KC - 1),
                    )
                yt = yst.tile([128, SQ], f32, tag="yt")
                nc.scalar.activation(yt, out, mybir.ActivationFunctionType.Identity,
                                     bias=bo_sb[:, do:do + 1])
                eng = (nc.sync, nc.scalar, nc.gpsimd)[do % 3]
                eng.dma_start(out=yT[do * 128:(do + 1) * 128, :], in_=yt[:, :])

    nc.finalize()
    return nc


def get_program(dbg=False):
    key = ("nc", dbg)
    if key not in _CACHE:
        _CACHE[key] = _build_program(dbg)
    return _CACHE[key]


def make_in_maps(x, Wq, bq, Wk, bk, Wv, bv, Wo, bo):
    """Host-side sharding: per-core input dicts."""
    import ml_dtypes

    bf16 = ml_dtypes.bfloat16
    x = np.ascontiguousarray(np.asarray(x, np.float32))
    wkb = np.concatenate([np.asarray(Wk, np.float32), np.asarray(bk, np.float32)[None]], 0)
    wvb = np.concatenate([np.asarray(Wv, np.float32), np.asarray(bv, np.float32)[None]], 0)
    # head permutation: device column-block p holds [head p | head p+8]
    perm = np.empty(DIM, np.int64)
    for p in range(8):
        perm[128 * p:128 * p + 64] = np.arange(64 * p, 64 * p + 64)
        perm[128 * p + 64:128 * p + 128] = np.arange(64 * (p + 8), 64 * (p + 8) + 64)
    wqp = np.asarray(Wq, np.float32)[:, perm]
    wop = np.asarray(Wo, np.float32)[perm, :]
    # [dd, part, kc, m] blocks so each dd's weights are one contiguous DMA
    wq_blk = np.ascontiguousarray(
        wqp.reshape(KC, 128, KC, 128).transpose(2, 1, 0, 3).astype(bf16))
    wo_blk = np.ascontiguousarray(
        wop.reshape(KC, 128, KC, 128).transpose(2, 1, 0, 3).astype(bf16))
    common = {
        "wq": wq_blk,
        "wk": np.ascontiguousarray(wkb.astype(bf16)),
        "wv": np.ascontiguousarray(wvb.astype(bf16)),
        "wo": wo_blk,
        "bq": np.ascontiguousarray(np.asarray(bq, np.float32)[perm].reshape(DIM, 1)),
        "bo": np.ascontiguousarray(np.asarray(bo, np.float32).reshape(DIM, 1)),
        "ident": np.eye(128, dtype=np.float32),
        "ones2": np.ones((128, G), bf16),
    }
    in_maps = []
    for c in range(NCORES):
        b, t = divmod(c, NCORES // BATCH)
        s0 = SQ * t
        xa = np.zeros((SK, DIM + 1), np.float32)
        lo, hi = max(0, s0 - HALF), min(SEQ, s0 + SQ + HALF)
        xa[lo - (s0 - HALF):hi - (s0 - HALF), :DIM] = x[b, lo:hi]
        xa[lo - (s0 - HALF):hi - (s0 - HALF), DIM] = 1.0
        in_maps.append({"xaT": np.ascontiguousarray(xa.T.astype(bf16)), **common})
    return in_maps


def assemble_output(results):
    y = np.empty((BATCH, SEQ, DIM), np.float32)
    for c in range(NCORES):
        b, t = divmod(c, NCORES // BATCH)
        y[b, SQ * t:SQ * (t + 1), :] = results[c]["yT"].T
    return y


def kernel(**inputs):
    from concourse.bass_utils import run_bass_kernel_spmd

    nc = get_program()
    in_maps = make_in_maps(**inputs)
    last_err = None
    for _ in range(3):  # retry: transient NRT device wedges recover on rerun
        try:
            res = run_bass_kernel_spmd(nc, in_maps, list(range(NCORES)))
            return assemble_output(res.results)
        except Exception as e:  # noqa: BLE001
            last_err = e
    raise last_err
